# revision 1
# baseline (speedup 1.0000x reference)
"""KAN layer (polynomial basis) TRN2 kernel.

out = gelu(sum_{i,k} x[b,i]^k * W[i,k,j] + bias[j]),  exact gelu.
B=4096, D=1024, K=5, U=1024, fp32 I/O.

Strategy:
  - Data-parallel over batch: 8 cores x 512 rows each.
  - k=0 term (x^0=1) constant-folded on host into the bias:
    bias_total = bias + sum_i W[i,0,:].
  - x is fed pre-transposed ([D, B_local]) so the contraction dim (D)
    lands on SBUF partitions; powers x^2,x^3,x^4 computed on-device (DVE).
  - Split-precision matmuls: every operand v = vh + vl with vh,vl bf16
    (16 mantissa bits total). out ~= xh@wh + xh@wl + xl@wh per term ->
    ~4e-6 relative error (fp32-class) at 3 bf16 matmuls per fp32 matmul
    (bf16 MM = 1 cyc/row vs fp32 = 4 cyc/row on TRN2 PE).
  - W hi/lo split + tiling done host-side (weights are pure inputs);
    x-power splits on device.
  - Output computed transposed ([U, B_local]) so the per-unit bias is a
    per-partition scalar, fused into the final Gelu activation; host
    transposes back during the gather.
"""

import os
import numpy as np
import ml_dtypes

from concourse import bacc
import concourse.mybir as mybir
import concourse.tile as tile
from concourse.bass_utils import run_bass_kernel_spmd

F32 = mybir.dt.float32
BF16 = mybir.dt.bfloat16
AF = mybir.ActivationFunctionType

NCORES = 8
B, D, K, U = 4096, 1024, 5, 1024
BL = B // NCORES  # 512 batch rows per core
ND = D // 128  # 8 d chunks
NU = U // 128  # 8 u chunks

LAST_EXEC_TIME_NS = None


def _build():
    nc = bacc.Bacc("TRN2", target_bir_lowering=False, debug=False)
    xt = nc.dram_tensor("xt", [D, BL], F32, kind="ExternalInput").ap()
    wblob = nc.dram_tensor(
        "wblob", [NU, ND, 128, 4 * 2 * 128], BF16, kind="ExternalInput"
    ).ap()
    bias2d = nc.dram_tensor("bias2d", [128, NU], F32, kind="ExternalInput").ap()
    out_t = nc.dram_tensor("out_t", [U, BL], F32, kind="ExternalOutput").ap()

    with tile.TileContext(nc) as tc:
        with (
            tc.tile_pool(name="xres", bufs=1) as xres,
            tc.tile_pool(name="tmp", bufs=2) as tmp,
            tc.tile_pool(name="wp", bufs=4) as wp,
            tc.tile_pool(name="op", bufs=2) as op,
            tc.tile_pool(name="ps", bufs=2, space="PSUM") as ps,
        ):
            bias_sb = xres.tile([128, NU], F32, name="bias_sb")
            nc.sync.dma_start(bias_sb, bias2d)

            # ---- powers + hi/lo splits, per d chunk (all resident) ----
            H = [[None] * ND for _ in range(4)]  # H[k][d], k: x^1..x^4
            L = [[None] * ND for _ in range(4)]
            for d in range(ND):
                xf = xres.tile([128, BL], F32, name=f"xf_{d}")
                nc.sync.dma_start(xf, xt[d * 128 : (d + 1) * 128, :])
                x2f = tmp.tile([128, BL], F32, name="x2f", tag="x2f")
                nc.vector.tensor_mul(out=x2f, in0=xf, in1=xf)
                x3f = tmp.tile([128, BL], F32, name="x3f", tag="x3f")
                nc.vector.tensor_mul(out=x3f, in0=x2f, in1=xf)
                x4f = tmp.tile([128, BL], F32, name="x4f", tag="x4f")
                nc.vector.tensor_mul(out=x4f, in0=x2f, in1=x2f)
                for k, src in enumerate([xf, x2f, x3f, x4f]):
                    h = xres.tile([128, BL], BF16, name=f"h{k}_{d}")
                    nc.vector.tensor_copy(h, src)
                    l = xres.tile([128, BL], BF16, name=f"l{k}_{d}")
                    nc.vector.tensor_sub(out=l, in0=src, in1=h)
                    H[k][d] = h
                    L[k][d] = l

            # ---- matmuls: out_T[u,:] = sum_{d,k} W[d,k,u].T @ x^k_T[d,:] ----
            for u in range(NU):
                pacc = ps.tile([128, BL], F32, name="pacc", tag="pacc")
                for d in range(ND):
                    wt = wp.tile([128, 4 * 2 * 128], BF16, name="wt", tag="wt")
                    nc.sync.dma_start(wt, wblob[u, d])
                    for k in range(4):
                        wh = wt[:, k * 256 : k * 256 + 128]
                        wl = wt[:, k * 256 + 128 : k * 256 + 256]
                        nc.tensor.matmul(
                            pacc, wh, H[k][d],
                            start=(d == 0 and k == 0), stop=False,
                        )
                        nc.tensor.matmul(pacc, wh, L[k][d], start=False, stop=False)
                        nc.tensor.matmul(
                            pacc, wl, H[k][d],
                            start=False, stop=(d == ND - 1 and k == 3),
                        )
                osb = op.tile([128, BL], F32, name="osb", tag="osb")
                nc.scalar.activation(
                    osb, pacc, AF.Gelu, bias=bias_sb[:, u : u + 1], scale=1.0
                )
                nc.sync.dma_start(out_t[u * 128 : (u + 1) * 128, :], osb)

    nc.compile()
    return nc


_NC_CACHE = None


def kernel(x, basis_weights, bias):
    global _NC_CACHE, LAST_EXEC_TIME_NS
    x = np.asarray(x, dtype=np.float32)
    W = np.asarray(basis_weights, dtype=np.float32)
    bias = np.asarray(bias, dtype=np.float32)

    # ---- host prep (layout only + constant folding of the x^0 term) ----
    xT = np.ascontiguousarray(x.T)  # (D, B)
    Wk = W[:, 1:5, :]  # (D, 4, U)
    wh = Wk.astype(ml_dtypes.bfloat16)
    wl = (Wk - wh.astype(np.float32)).astype(ml_dtypes.bfloat16)
    st = np.stack([wh, wl], axis=2)  # (D, 4, 2, U)
    blob = st.reshape(ND, 128, 4, 2, NU, 128).transpose(4, 0, 1, 2, 3, 5)
    blob = np.ascontiguousarray(blob.reshape(NU, ND, 128, 4 * 2 * 128))
    bias_total = (
        bias.astype(np.float64) + W[:, 0, :].astype(np.float64).sum(axis=0)
    ).astype(np.float32)
    bias2d = np.ascontiguousarray(bias_total.reshape(NU, 128).T)

    in_maps = []
    for i in range(NCORES):
        xt_i = np.ascontiguousarray(xT[:, i * BL : (i + 1) * BL])
        in_maps.append({"xt": xt_i, "wblob": blob, "bias2d": bias2d})

    if _NC_CACHE is None:
        _NC_CACHE = _build()
    nc = _NC_CACHE

    trace = bool(os.environ.get("KERNEL_TRACE"))
    res = run_bass_kernel_spmd(
        nc, in_maps, core_ids=list(range(NCORES)), trace=trace
    )
    LAST_EXEC_TIME_NS = res.exec_time_ns

    out = np.empty((B, U), dtype=np.float32)
    for i in range(NCORES):
        out[i * BL : (i + 1) * BL, :] = res.results[i]["out_t"].T
    return out



# revision 2
# speedup vs baseline: 2.7644x; 2.7644x over previous
"""KAN layer (polynomial basis) TRN2 kernel.

out = gelu(sum_{i,k} x[b,i]^k * W[i,k,j] + bias[j]),  exact gelu.
B=4096, D=1024, K=5, U=1024, fp32 I/O.

Strategy:
  - Data-parallel over batch: 8 cores x 512 rows each.
  - k=0 term (x^0=1) constant-folded on host into the bias:
    bias_total = bias + sum_i W[i,0,:].
  - x fed pre-transposed ([D, B_local]) so the contraction dim (D) lands
    on SBUF partitions; powers x^2..x^4 computed on-device.
  - Mixed-precision matmuls sized to the 2e-2 rel-err budget:
      k=1,2: both operands fp8 e4m3, fused into ONE DoubleRow matmul per
             (d,u) tile (2 fp8 weights/cell -> 0.5 cyc/row).
      k=3,4: both operands fp16 (1 cyc/row, 11-bit mantissa; bf16's 8
             bits would also pass but fp16 is free at the same speed).
    Weights pre-scaled by S=256 on host so fp8 W stays in e4m3 normal
    range; undone by the activation's scale=1/S.
  - Loop order d-outer/u-inner with all 8 PSUM banks as accumulators, so
    the PE consumes each x chunk right after its prep and never waits on
    a full prep pass.
  - Output computed transposed ([U, B_local]) so the per-unit bias is a
    per-partition scalar fused into the final Gelu; host transposes back.
"""

import os
import numpy as np
import ml_dtypes

from concourse import bacc
import concourse.mybir as mybir
import concourse.tile as tile
from concourse.bass_utils import run_bass_kernel_spmd

F32 = mybir.dt.float32
F16 = mybir.dt.float16
F8 = mybir.dt.float8e4
AF = mybir.ActivationFunctionType
DR = mybir.MatmulPerfMode.DoubleRow

NCORES = 8
B, D, K, U = 4096, 1024, 5, 1024
BL = B // NCORES  # 512 batch rows per core
ND = D // 128  # 8 d chunks
NU = U // 128  # 8 u chunks
S = 256.0  # weight pre-scale (fp8 subnormal avoidance); undone in gelu

LAST_EXEC_TIME_NS = None


def _build():
    nc = bacc.Bacc("TRN2", target_bir_lowering=False, debug=False)
    xt = nc.dram_tensor("xt", [D, BL], F32, kind="ExternalInput").ap()
    w12 = nc.dram_tensor("w12", [ND, 128, 2, U], F8, kind="ExternalInput").ap()
    w3 = nc.dram_tensor("w3", [ND, 128, U], F16, kind="ExternalInput").ap()
    w4 = nc.dram_tensor("w4", [ND, 128, U], F16, kind="ExternalInput").ap()
    bias2d = nc.dram_tensor("bias2d", [128, NU], F32, kind="ExternalInput").ap()
    out_t = nc.dram_tensor("out_t", [U, BL], F32, kind="ExternalOutput").ap()

    with tile.TileContext(nc) as tc:
        with (
            tc.tile_pool(name="xin", bufs=2) as xin,
            tc.tile_pool(name="wp", bufs=3) as wp,
            tc.tile_pool(name="xk", bufs=2) as xk,
            tc.tile_pool(name="tmp", bufs=2) as tmp,
            tc.tile_pool(name="op", bufs=2) as op,
            tc.tile_pool(name="res", bufs=1) as res,
            tc.tile_pool(name="ps", bufs=1, space="PSUM") as ps,
        ):
            bias_sb = res.tile([128, NU], F32, name="bias_sb")
            nc.sync.dma_start(bias_sb, bias2d)

            pacc = [
                ps.tile([128, BL], F32, name=f"pacc{u}", tag=f"pacc{u}")
                for u in range(NU)
            ]

            for d in range(ND):
                xf = xin.tile([128, BL], F32, name="xf", tag="xf")
                nc.sync.dma_start(xf, xt[d * 128 : (d + 1) * 128, :])
                w12t = wp.tile([128, 2, U], F8, name="w12t", tag="w12t")
                nc.sync.dma_start(w12t, w12[d])
                w3t = wp.tile([128, U], F16, name="w3t", tag="w3t")
                nc.sync.dma_start(w3t, w3[d])
                w4t = wp.tile([128, U], F16, name="w4t", tag="w4t")
                nc.sync.dma_start(w4t, w4[d])

                # powers: x^2 on ACT, x^3 on DVE, x^4 on GPSIMD
                x2f = tmp.tile([128, BL], F32, name="x2f", tag="x2f")
                nc.scalar.activation(x2f, xf, AF.Square)
                x3f = tmp.tile([128, BL], F32, name="x3f", tag="x3f")
                nc.vector.tensor_mul(out=x3f, in0=x2f, in1=xf)
                x4f = tmp.tile([128, BL], F32, name="x4f", tag="x4f")
                nc.gpsimd.tensor_mul(out=x4f, in0=x2f, in1=x2f)

                xq = xk.tile([128, 2, BL], F8, name="xq", tag="xq")
                nc.vector.tensor_copy(xq[:, 0, :], xf)
                nc.vector.tensor_copy(xq[:, 1, :], x2f)
                x3h = xk.tile([128, BL], F16, name="x3h", tag="x3h")
                nc.vector.tensor_copy(x3h, x3f)
                x4h = xk.tile([128, BL], F16, name="x4h", tag="x4h")
                nc.vector.tensor_copy(x4h, x4f)

                last = d == ND - 1
                for u in range(NU):
                    nc.tensor.matmul(
                        pacc[u],
                        w12t[:, :, u * 128 : (u + 1) * 128],
                        xq,
                        start=(d == 0),
                        stop=False,
                        perf_mode=DR,
                    )
                    nc.tensor.matmul(
                        pacc[u],
                        w3t[:, u * 128 : (u + 1) * 128],
                        x3h,
                        start=False,
                        stop=False,
                    )
                    nc.tensor.matmul(
                        pacc[u],
                        w4t[:, u * 128 : (u + 1) * 128],
                        x4h,
                        start=False,
                        stop=last,
                    )
                    if last:
                        osb = op.tile([128, BL], F32, name="osb", tag="osb")
                        nc.scalar.activation(
                            osb,
                            pacc[u],
                            AF.Gelu,
                            bias=bias_sb[:, u : u + 1],
                            scale=1.0 / S,
                        )
                        nc.sync.dma_start(out_t[u * 128 : (u + 1) * 128, :], osb)

    nc.compile()
    return nc


_NC_CACHE = None


def kernel(x, basis_weights, bias):
    global _NC_CACHE, LAST_EXEC_TIME_NS
    x = np.asarray(x, dtype=np.float32)
    W = np.asarray(basis_weights, dtype=np.float32)
    bias = np.asarray(bias, dtype=np.float32)

    # ---- host prep: layout + dtype split + k=0 constant folding ----
    xT = np.ascontiguousarray(x.T)  # (D, B)
    Ws = W * np.float32(S)
    w12b = np.stack([Ws[:, 1, :], Ws[:, 2, :]], axis=1)  # (D, 2, U)
    w12b = np.ascontiguousarray(
        w12b.reshape(ND, 128, 2, U).astype(ml_dtypes.float8_e4m3)
    )
    w3b = np.ascontiguousarray(Ws[:, 3, :].reshape(ND, 128, U).astype(np.float16))
    w4b = np.ascontiguousarray(Ws[:, 4, :].reshape(ND, 128, U).astype(np.float16))
    bias_total = (
        bias.astype(np.float64) + W[:, 0, :].astype(np.float64).sum(axis=0)
    ).astype(np.float32)
    bias2d = np.ascontiguousarray(bias_total.reshape(NU, 128).T)

    in_maps = []
    for i in range(NCORES):
        xt_i = np.ascontiguousarray(xT[:, i * BL : (i + 1) * BL])
        in_maps.append(
            {"xt": xt_i, "w12": w12b, "w3": w3b, "w4": w4b, "bias2d": bias2d}
        )

    if _NC_CACHE is None:
        _NC_CACHE = _build()
    nc = _NC_CACHE

    trace = bool(os.environ.get("KERNEL_TRACE"))
    res = run_bass_kernel_spmd(
        nc, in_maps, core_ids=list(range(NCORES)), trace=trace
    )
    LAST_EXEC_TIME_NS = res.exec_time_ns

    out = np.empty((B, U), dtype=np.float32)
    for i in range(NCORES):
        out[i * BL : (i + 1) * BL, :] = res.results[i]["out_t"].T
    return out


# revision 7
# speedup vs baseline: 2.9429x; 1.0646x over previous
"""KAN layer (polynomial basis) TRN2 kernel.

out = gelu(sum_{i,k} x[b,i]^k * W[i,k,j] + bias[j]),  exact gelu.
B=4096, D=1024, K=5, U=1024, fp32 I/O.

Strategy:
  - Data-parallel over batch: 8 cores x 512 rows each.
  - k=0 term (x^0=1) constant-folded on host into the bias:
    bias_total = bias + sum_i W[i,0,:].
  - x fed pre-transposed ([D, B_local]) so the contraction dim (D) lands
    on SBUF partitions; powers x^2..x^4 computed on-device.
  - Mixed-precision matmuls sized to the 2e-2 rel-err budget:
      k=1,2: both operands fp8 e4m3, fused into ONE DoubleRow matmul per
             (d,u) tile (2 fp8 weights/cell -> 0.5 cyc/row).
      k=3,4: both operands fp16 (1 cyc/row, 11-bit mantissa; bf16's 8
             bits would also pass but fp16 is free at the same speed).
    Weights pre-scaled by S=256 on host so fp8 W stays in e4m3 normal
    range; undone by the activation's scale=1/S.
  - Loop order d-outer/u-inner with all 8 PSUM banks as accumulators, so
    the PE consumes each x chunk right after its prep and never waits on
    a full prep pass.
  - Output computed transposed ([U, B_local]) so the per-unit bias is a
    per-partition scalar fused into the final Gelu; host transposes back.
"""

import os
import numpy as np
import ml_dtypes

from concourse import bacc
import concourse.mybir as mybir
import concourse.tile as tile
from concourse.bass_utils import run_bass_kernel_spmd

F32 = mybir.dt.float32
F16 = mybir.dt.float16
F8 = mybir.dt.float8e4
AF = mybir.ActivationFunctionType
DR = mybir.MatmulPerfMode.DoubleRow

NCORES = 8
B, D, K, U = 4096, 1024, 5, 1024
BL = B // NCORES  # 512 batch rows per core
ND = D // 128  # 8 d chunks
NU = U // 128  # 8 u chunks
S = 256.0  # weight pre-scale (fp8 subnormal avoidance); undone in gelu

LAST_EXEC_TIME_NS = None


def _build():
    nc = bacc.Bacc("TRN2", target_bir_lowering=False, debug=False)
    xt = nc.dram_tensor("xt", [D, BL], F32, kind="ExternalInput").ap()
    w12 = nc.dram_tensor("w12", [ND, 128, 2, U], F8, kind="ExternalInput").ap()
    w34 = nc.dram_tensor("w34", [ND, 128, 2 * U], F16, kind="ExternalInput").ap()
    bias2d = nc.dram_tensor("bias2d", [128, NU], F32, kind="ExternalInput").ap()
    out_t = nc.dram_tensor("out_t", [U, BL], F32, kind="ExternalOutput").ap()

    with tile.TileContext(nc) as tc:
        with (
            tc.tile_pool(name="xin", bufs=2) as xin,
            tc.tile_pool(name="wp", bufs=3) as wp,
            tc.tile_pool(name="xk", bufs=2) as xk,
            tc.tile_pool(name="tmp", bufs=2) as tmp,
            tc.tile_pool(name="op", bufs=4) as op,
            tc.tile_pool(name="res", bufs=1) as res,
            tc.tile_pool(name="ps", bufs=1, space="PSUM") as ps,
        ):
            pacc = [
                ps.tile([128, BL], F32, name=f"pacc{u}", tag=f"pacc{u}")
                for u in range(NU)
            ]

            bias_sb = res.tile([128, NU], F32, name="bias_sb")

            for d in range(ND):
                # x on the scalar-engine HWDGE queue, weights on sync's —
                # parallel DMA issue (600ns per descriptor-gen each).
                xf = xin.tile([128, BL], F32, name="xf", tag="xf")
                nc.scalar.dma_start(xf, xt[d * 128 : (d + 1) * 128, :])
                w12t = wp.tile([128, 2, U], F8, name="w12t", tag="w12t")
                nc.sync.dma_start(w12t, w12[d])
                w34t = wp.tile([128, 2 * U], F16, name="w34t", tag="w34t")
                nc.sync.dma_start(w34t, w34[d])
                if d == 0:
                    nc.scalar.dma_start(bias_sb, bias2d)

                # prep: ACT does x^2 (f32) and x^4=(x^2)^2 (straight to f16);
                # DVE does the fp8 casts, x^3 mul and its cast.
                x2f = tmp.tile([128, BL], F32, name="x2f", tag="x2f")
                nc.scalar.activation(x2f, xf, AF.Square)
                xq = xk.tile([128, 2, BL], F8, name="xq", tag="xq")
                nc.vector.tensor_copy(xq[:, 0, :], xf)
                nc.vector.tensor_copy(xq[:, 1, :], x2f)
                x4h = xk.tile([128, BL], F16, name="x4h", tag="x4h")
                nc.scalar.activation(x4h, x2f, AF.Square)
                x3f = tmp.tile([128, BL], F32, name="x3f", tag="x3f")
                nc.vector.tensor_mul(out=x3f, in0=x2f, in1=xf)
                x3h = xk.tile([128, BL], F16, name="x3h", tag="x3h")
                nc.vector.tensor_copy(x3h, x3f)

                last = d == ND - 1
                for u in range(NU):
                    nc.tensor.matmul(
                        pacc[u],
                        w12t[:, :, u * 128 : (u + 1) * 128],
                        xq,
                        start=(d == 0),
                        stop=False,
                        perf_mode=DR,
                    )
                    nc.tensor.matmul(
                        pacc[u],
                        w34t[:, u * 128 : (u + 1) * 128],
                        x3h,
                        start=False,
                        stop=False,
                    )
                    nc.tensor.matmul(
                        pacc[u],
                        w34t[:, U + u * 128 : U + (u + 1) * 128],
                        x4h,
                        start=False,
                        stop=last,
                    )
                    if last:
                        osb = op.tile([128, BL], F32, name="osb", tag="osb")
                        nc.scalar.activation(
                            osb,
                            pacc[u],
                            AF.Gelu,
                            bias=bias_sb[:, u : u + 1],
                            scale=1.0 / S,
                        )
                        nc.sync.dma_start(out_t[u * 128 : (u + 1) * 128, :], osb)

    nc.compile()
    return nc


_NC_CACHE = None


def kernel(x, basis_weights, bias):
    global _NC_CACHE, LAST_EXEC_TIME_NS
    x = np.asarray(x, dtype=np.float32)
    W = np.asarray(basis_weights, dtype=np.float32)
    bias = np.asarray(bias, dtype=np.float32)

    # ---- host prep: layout + dtype split + k=0 constant folding ----
    xT = np.ascontiguousarray(x.T)  # (D, B)
    Ws = W * np.float32(S)
    w12b = np.stack([Ws[:, 1, :], Ws[:, 2, :]], axis=1)  # (D, 2, U)
    w12b = np.ascontiguousarray(
        w12b.reshape(ND, 128, 2, U).astype(ml_dtypes.float8_e4m3)
    )
    w34b = np.concatenate([Ws[:, 3, :], Ws[:, 4, :]], axis=1)  # (D, 2U)
    w34b = np.ascontiguousarray(w34b.reshape(ND, 128, 2 * U).astype(np.float16))
    bias_total = (
        bias.astype(np.float64) + W[:, 0, :].astype(np.float64).sum(axis=0)
    ).astype(np.float32)
    bias2d = np.ascontiguousarray(bias_total.reshape(NU, 128).T)

    in_maps = []
    for i in range(NCORES):
        xt_i = np.ascontiguousarray(xT[:, i * BL : (i + 1) * BL])
        in_maps.append({"xt": xt_i, "w12": w12b, "w34": w34b, "bias2d": bias2d})

    if _NC_CACHE is None:
        _NC_CACHE = _build()
    nc = _NC_CACHE

    trace = bool(os.environ.get("KERNEL_TRACE"))
    res = run_bass_kernel_spmd(
        nc, in_maps, core_ids=list(range(NCORES)), trace=trace
    )
    LAST_EXEC_TIME_NS = res.exec_time_ns

    out = np.empty((B, U), dtype=np.float32)
    for i in range(NCORES):
        out[i * BL : (i + 1) * BL, :] = res.results[i]["out_t"].T
    return out


# revision 11
# speedup vs baseline: 3.1139x; 1.0581x over previous
"""KAN layer (polynomial basis) TRN2 kernel.

out = gelu(sum_{i,k} x[b,i]^k * W[i,k,j] + bias[j]),  exact gelu.
B=4096, D=1024, K=5, U=1024, fp32 I/O.

Strategy:
  - Data-parallel over batch: 8 cores x 512 rows each.
  - k=0 term (x^0=1) constant-folded on host into the bias:
    bias_total = bias + sum_i W[i,0,:].
  - x fed pre-transposed ([D, B_local]) so the contraction dim (D) lands
    on SBUF partitions; powers x^2..x^4 computed on-device.
  - Mixed-precision matmuls sized to the 2e-2 rel-err budget:
      k=1,2: both operands fp8 e4m3, fused into ONE DoubleRow matmul per
             (d,u) tile (2 fp8 weights/cell -> 0.5 cyc/row).
      k=3,4: both operands fp16 (1 cyc/row, 11-bit mantissa; bf16's 8
             bits would also pass but fp16 is free at the same speed).
    Weights pre-scaled by S=256 on host so fp8 W stays in e4m3 normal
    range; undone by the activation's scale=1/S.
  - Loop order d-outer/u-inner with all 8 PSUM banks as accumulators, so
    the PE consumes each x chunk right after its prep and never waits on
    a full prep pass.
  - Output computed transposed ([U, B_local]) so the per-unit bias is a
    per-partition scalar fused into the final Gelu; host transposes back.
"""

import os
import numpy as np
import ml_dtypes

from concourse import bacc
import concourse.mybir as mybir
import concourse.tile as tile
from concourse.bass_utils import run_bass_kernel_spmd

F32 = mybir.dt.float32
F16 = mybir.dt.float16
F8 = mybir.dt.float8e4
AF = mybir.ActivationFunctionType
DR = mybir.MatmulPerfMode.DoubleRow

NCORES = 8
B, D, K, U = 4096, 1024, 5, 1024
BL = B // NCORES  # 512 batch rows per core
ND = D // 128  # 8 d chunks
NU = U // 128  # 8 u chunks
S = 256.0  # weight pre-scale (fp8 subnormal avoidance); undone in gelu

LAST_EXEC_TIME_NS = None


def _build():
    nc = bacc.Bacc("TRN2", target_bir_lowering=False, debug=False)
    xt = nc.dram_tensor("xt", [D, BL], F32, kind="ExternalInput").ap()
    w12 = nc.dram_tensor("w12", [ND, 128, 2, U], F8, kind="ExternalInput").ap()
    w3 = nc.dram_tensor("w3", [ND, 128, U], F16, kind="ExternalInput").ap()
    w4 = nc.dram_tensor("w4", [ND, 128, U], F16, kind="ExternalInput").ap()
    bias2d = nc.dram_tensor("bias2d", [128, NU], F32, kind="ExternalInput").ap()
    out_t = nc.dram_tensor("out_t", [U, BL], F32, kind="ExternalOutput").ap()

    with tile.TileContext(nc) as tc:
        with (
            tc.tile_pool(name="xin", bufs=2) as xin,
            tc.tile_pool(name="wp", bufs=3) as wp,
            tc.tile_pool(name="xk", bufs=2) as xk,
            tc.tile_pool(name="tmp", bufs=2) as tmp,
            tc.tile_pool(name="op", bufs=4) as op,
            tc.tile_pool(name="res", bufs=1) as res,
            tc.tile_pool(name="ps", bufs=1, space="PSUM") as ps,
        ):
            pacc = [
                ps.tile([128, BL], F32, name=f"pacc{u}", tag=f"pacc{u}")
                for u in range(NU)
            ]

            bias_sb = res.tile([128, NU], F32, name="bias_sb")
            nc.sync.dma_start(bias_sb, bias2d)

            for d in range(ND):
                # x on the scalar-engine HWDGE queue, weights on sync's —
                # parallel DMA issue (600ns per descriptor-gen each).
                xf = xin.tile([128, BL], F32, name="xf", tag="xf")
                nc.scalar.dma_start(xf, xt[d * 128 : (d + 1) * 128, :])
                w12t = wp.tile([128, 2, U], F8, name="w12t", tag="w12t")
                nc.sync.dma_start(w12t, w12[d])
                w3t = wp.tile([128, U], F16, name="w3t", tag="w3t")
                nc.sync.dma_start(w3t, w3[d])
                w4t = wp.tile([128, U], F16, name="w4t", tag="w4t")
                nc.sync.dma_start(w4t, w4[d])

                # prep: DVE alone feeds the DR matmul (xq from xf only, no
                # cross-engine dep); ACT squares feed the fp16 side.
                xq = xk.tile([128, 2, BL], F8, name="xq", tag="xq")
                nc.vector.tensor_copy(xq[:, 0, :], xf)
                nc.vector.tensor_mul(out=xq[:, 1, :], in0=xf, in1=xf)
                x2f = tmp.tile([128, BL], F32, name="x2f", tag="x2f")
                nc.scalar.activation(x2f, xf, AF.Square)
                x3h = xk.tile([128, BL], F16, name="x3h", tag="x3h")
                nc.vector.tensor_mul(out=x3h, in0=x2f, in1=xf)
                x4h = xk.tile([128, BL], F16, name="x4h", tag="x4h")
                nc.scalar.activation(x4h, x2f, AF.Square)

                last = d == ND - 1
                # d=0: all DR matmuls first so the fp16 ones have time for
                # w3/w4's first DMA transfers to land.
                if d == 0:
                    for u in range(NU):
                        nc.tensor.matmul(
                            pacc[u],
                            w12t[:, :, u * 128 : (u + 1) * 128],
                            xq,
                            start=True,
                            stop=False,
                            perf_mode=DR,
                        )
                    for u in range(NU):
                        nc.tensor.matmul(
                            pacc[u],
                            w3t[:, u * 128 : (u + 1) * 128],
                            x3h,
                            start=False,
                            stop=False,
                        )
                        nc.tensor.matmul(
                            pacc[u],
                            w4t[:, u * 128 : (u + 1) * 128],
                            x4h,
                            start=False,
                            stop=False,
                        )
                    continue
                for u in range(NU):
                    nc.tensor.matmul(
                        pacc[u],
                        w12t[:, :, u * 128 : (u + 1) * 128],
                        xq,
                        start=False,
                        stop=False,
                        perf_mode=DR,
                    )
                    nc.tensor.matmul(
                        pacc[u],
                        w3t[:, u * 128 : (u + 1) * 128],
                        x3h,
                        start=False,
                        stop=False,
                    )
                    nc.tensor.matmul(
                        pacc[u],
                        w4t[:, u * 128 : (u + 1) * 128],
                        x4h,
                        start=False,
                        stop=last,
                    )
                    if last:
                        osb = op.tile([128, BL], F32, name="osb", tag="osb")
                        nc.scalar.activation(
                            osb,
                            pacc[u],
                            AF.Gelu,
                            bias=bias_sb[:, u : u + 1],
                            scale=1.0 / S,
                        )
                        # split the store across both HWDGE queues so the
                        # two halves transfer in parallel at the tail
                        nc.sync.dma_start(
                            out_t[u * 128 : (u + 1) * 128, : BL // 2],
                            osb[:, : BL // 2],
                        )
                        nc.scalar.dma_start(
                            out_t[u * 128 : (u + 1) * 128, BL // 2 :],
                            osb[:, BL // 2 :],
                        )

    nc.compile()
    return nc


_NC_CACHE = None


def kernel(x, basis_weights, bias):
    global _NC_CACHE, LAST_EXEC_TIME_NS
    x = np.asarray(x, dtype=np.float32)
    W = np.asarray(basis_weights, dtype=np.float32)
    bias = np.asarray(bias, dtype=np.float32)

    # ---- host prep: layout + dtype split + k=0 constant folding ----
    xT = np.ascontiguousarray(x.T)  # (D, B)
    Ws = W * np.float32(S)
    w12b = np.stack([Ws[:, 1, :], Ws[:, 2, :]], axis=1)  # (D, 2, U)
    w12b = np.ascontiguousarray(
        w12b.reshape(ND, 128, 2, U).astype(ml_dtypes.float8_e4m3)
    )
    w3b = np.ascontiguousarray(Ws[:, 3, :].reshape(ND, 128, U).astype(np.float16))
    w4b = np.ascontiguousarray(Ws[:, 4, :].reshape(ND, 128, U).astype(np.float16))
    bias_total = (
        bias.astype(np.float64) + W[:, 0, :].astype(np.float64).sum(axis=0)
    ).astype(np.float32)
    bias2d = np.ascontiguousarray(bias_total.reshape(NU, 128).T)

    in_maps = []
    for i in range(NCORES):
        xt_i = np.ascontiguousarray(xT[:, i * BL : (i + 1) * BL])
        in_maps.append(
            {"xt": xt_i, "w12": w12b, "w3": w3b, "w4": w4b, "bias2d": bias2d}
        )

    if _NC_CACHE is None:
        _NC_CACHE = _build()
    nc = _NC_CACHE

    trace = bool(os.environ.get("KERNEL_TRACE"))
    res = run_bass_kernel_spmd(
        nc, in_maps, core_ids=list(range(NCORES)), trace=trace
    )
    LAST_EXEC_TIME_NS = res.exec_time_ns

    out = np.empty((B, U), dtype=np.float32)
    for i in range(NCORES):
        out[i * BL : (i + 1) * BL, :] = res.results[i]["out_t"].T
    return out


# revision 17
# speedup vs baseline: 3.4985x; 1.1235x over previous
"""KAN layer (polynomial basis) TRN2 kernel.

out = gelu(sum_{i,k} x[b,i]^k * W[i,k,j] + bias[j]),  exact gelu.
B=4096, D=1024, K=5, U=1024, fp32 I/O.

Strategy:
  - Data-parallel over batch: 8 cores x 512 rows each.
  - k=0 term (x^0=1) constant-folded on host into the bias:
    bias_total = bias + sum_i W[i,0,:].
  - x fed pre-transposed ([D, B_local]) so the contraction dim (D) lands
    on SBUF partitions; powers x^2..x^4 computed on-device.
  - Mixed-precision matmuls sized to the 2e-2 rel-err budget:
      k=1,2: both operands fp8 e4m3, fused into ONE DoubleRow matmul per
             (d,u) tile (2 fp8 weights/cell -> 0.5 cyc/row).
      k=3:   both operands fp8, with TWO ADJACENT d-chunks fused into one
             DoubleRow matmul (the pair rides the two DR slots), halving
             the k=3 cost again. x^3 is scaled by 1/2 so its e4m3 image
             stays under TRN's 240 cap (w3 scaled by 2S to compensate).
      k=4:   both operands fp16 (1 cyc/row; 11-bit mantissa keeps the
             x^4-term error negligible - it has the widest dynamic range).
    Weights pre-scaled by S=256 on host so fp8 W stays in e4m3 normal
    range; undone by the activation's scale=1/S.
    Measured rel err ~1.3e-2 vs the 2e-2 gate (dominated by the single-
    fp8 x^3 term; k=4 in fp8 as well would blow the budget).
  - Loop order d-outer/u-inner with all 8 PSUM banks as accumulators, so
    the PE consumes each x chunk right after its prep and never waits on
    a full prep pass.
  - Output computed transposed ([U, B_local]) so the per-unit bias is a
    per-partition scalar fused into the final Gelu; host transposes back.
"""

import os
import numpy as np
import ml_dtypes

from concourse import bacc
import concourse.mybir as mybir
import concourse.tile as tile
from concourse.bass_utils import run_bass_kernel_spmd

F32 = mybir.dt.float32
F16 = mybir.dt.float16
F8 = mybir.dt.float8e4
AF = mybir.ActivationFunctionType
DR = mybir.MatmulPerfMode.DoubleRow
MUL = mybir.AluOpType.mult

NCORES = 8
B, D, K, U = 4096, 1024, 5, 1024
BL = B // NCORES  # 512 batch rows per core
ND = D // 128  # 8 d chunks
NU = U // 128  # 8 u chunks
S = 256.0  # weight pre-scale (fp8 subnormal avoidance); undone in gelu

LAST_EXEC_TIME_NS = None


def _build():
    nc = bacc.Bacc("TRN2", target_bir_lowering=False, debug=False)
    xt = nc.dram_tensor("xt", [D, BL], F32, kind="ExternalInput").ap()
    w12 = nc.dram_tensor("w12", [ND, 128, 2, U], F8, kind="ExternalInput").ap()
    w3q = nc.dram_tensor("w3q", [ND // 2, 128, 2, U], F8, kind="ExternalInput").ap()
    w4 = nc.dram_tensor("w4", [ND, 128, U], F16, kind="ExternalInput").ap()
    bias2d = nc.dram_tensor("bias2d", [128, NU], F32, kind="ExternalInput").ap()
    out_t = nc.dram_tensor("out_t", [U, BL], F32, kind="ExternalOutput").ap()

    with tile.TileContext(nc) as tc:
        with (
            tc.tile_pool(name="xin", bufs=2) as xin,
            tc.tile_pool(name="wp", bufs=3) as wp,
            tc.tile_pool(name="xk", bufs=2) as xk,
            tc.tile_pool(name="tmp", bufs=2) as tmp,
            tc.tile_pool(name="op", bufs=4) as op,
            tc.tile_pool(name="res", bufs=1) as res,
            tc.tile_pool(name="ps", bufs=1, space="PSUM") as ps,
        ):
            pacc = [
                ps.tile([128, BL], F32, name=f"pacc{u}", tag=f"pacc{u}")
                for u in range(NU)
            ]

            bias_sb = res.tile([128, NU], F32, name="bias_sb")
            nc.sync.dma_start(bias_sb, bias2d)

            x4h_prev = None
            for d in range(ND):
                p, sl = d // 2, d % 2
                last = d == ND - 1
                # x on the scalar-engine HWDGE queue, weights on sync's —
                # parallel DMA issue (600ns per descriptor-gen each). The
                # very first x chunk is split across both queues to halve
                # its transfer latency (it gates the whole prolog).
                xf = xin.tile([128, BL], F32, name="xf", tag="xf")
                if d == 0:
                    nc.sync.dma_start(
                        xf[:, : BL // 2], xt[0:128, : BL // 2]
                    )
                    nc.scalar.dma_start(
                        xf[:, BL // 2 :], xt[0:128, BL // 2 :]
                    )
                else:
                    nc.scalar.dma_start(xf, xt[d * 128 : (d + 1) * 128, :])
                w12t = wp.tile([128, 2, U], F8, name="w12t", tag="w12t")
                nc.sync.dma_start(w12t, w12[d])
                if sl == 0:
                    w4ta = wp.tile([128, U], F16, name="w4ta", tag="w4ta")
                    nc.sync.dma_start(w4ta, w4[d])
                    w3qt = wp.tile([128, 2, U], F8, name="w3qt", tag="w3qt")
                    nc.sync.dma_start(w3qt, w3q[p])
                else:
                    w4tb = wp.tile([128, U], F16, name="w4tb", tag="w4tb")
                    nc.sync.dma_start(w4tb, w4[d])

                # prep: DVE alone feeds the DR matmul (xq from xf only, no
                # cross-engine dep); ACT squares feed x^3 (fp8 pair slot)
                # and x^4 (fp16).
                xq = xk.tile([128, 2, BL], F8, name="xq", tag="xq")
                nc.vector.tensor_copy(xq[:, 0, :], xf)
                nc.vector.tensor_mul(out=xq[:, 1, :], in0=xf, in1=xf)
                x2f = tmp.tile([128, BL], F32, name="x2f", tag="x2f")
                nc.scalar.activation(x2f, xf, AF.Square)
                if sl == 0:
                    x3qp = xk.tile([128, 2, BL], F8, name="x3q", tag="x3q")
                nc.vector.scalar_tensor_tensor(
                    out=x3qp[:, sl, :], in0=x2f, scalar=0.5, in1=xf,
                    op0=MUL, op1=MUL,
                )
                x4h = xk.tile([128, BL], F16, name="x4h", tag="x4h")
                nc.scalar.activation(x4h, x2f, AF.Square)

                def mm_dr(u, wt, rhs, start=False, stop=False):
                    nc.tensor.matmul(
                        pacc[u], wt[:, :, u * 128 : (u + 1) * 128], rhs,
                        start=start, stop=stop, perf_mode=DR,
                    )

                def mm16(u, wt, rhs, stop=False):
                    nc.tensor.matmul(
                        pacc[u], wt[:, u * 128 : (u + 1) * 128], rhs,
                        start=False, stop=stop,
                    )

                if sl == 0:
                    # even chunk: just its k1/k2 DR matmuls; k3 waits for
                    # the pair, k4 waits a chunk so w4's DMA can land.
                    for u in range(NU):
                        mm_dr(u, w12t, xq, start=(d == 0))
                    x4h_prev, w4_prev = x4h, w4ta
                elif not last:
                    for u in range(NU):
                        mm16(u, w4_prev, x4h_prev)
                    for u in range(NU):
                        mm_dr(u, w12t, xq)
                    for u in range(NU):
                        mm_dr(u, w3qt, x3qp)
                    for u in range(NU):
                        mm16(u, w4tb, x4h)
                else:
                    # final chunk: interleave per-u so each accumulator
                    # stops early enough for its gelu+store to pipeline.
                    for u in range(NU):
                        mm16(u, w4_prev, x4h_prev)
                    for u in range(NU):
                        mm_dr(u, w12t, xq)
                        mm_dr(u, w3qt, x3qp)
                        mm16(u, w4tb, x4h, stop=True)
                        osb = op.tile([128, BL], F32, name="osb", tag="osb")
                        nc.scalar.activation(
                            osb, pacc[u], AF.Gelu,
                            bias=bias_sb[:, u : u + 1], scale=1.0 / S,
                        )
                        # split the store across both HWDGE queues so the
                        # two halves transfer in parallel at the tail
                        nc.sync.dma_start(
                            out_t[u * 128 : (u + 1) * 128, : BL // 2],
                            osb[:, : BL // 2],
                        )
                        nc.scalar.dma_start(
                            out_t[u * 128 : (u + 1) * 128, BL // 2 :],
                            osb[:, BL // 2 :],
                        )

    nc.compile()
    return nc


_NC_CACHE = None


def kernel(x, basis_weights, bias):
    global _NC_CACHE, LAST_EXEC_TIME_NS
    x = np.asarray(x, dtype=np.float32)
    W = np.asarray(basis_weights, dtype=np.float32)
    bias = np.asarray(bias, dtype=np.float32)

    # ---- host prep: layout + dtype split + k=0 constant folding ----
    xT = np.ascontiguousarray(x.T)  # (D, B)
    Ws = W * np.float32(S)
    w12b = np.stack([Ws[:, 1, :], Ws[:, 2, :]], axis=1)  # (D, 2, U)
    w12b = np.ascontiguousarray(
        w12b.reshape(ND, 128, 2, U).astype(ml_dtypes.float8_e4m3)
    )
    w3qb = (2.0 * Ws[:, 3, :]).reshape(ND // 2, 2, 128, U).transpose(0, 2, 1, 3)
    w3qb = np.ascontiguousarray(w3qb.astype(ml_dtypes.float8_e4m3))
    w4b = np.ascontiguousarray(Ws[:, 4, :].reshape(ND, 128, U).astype(np.float16))
    bias_total = (
        bias.astype(np.float64) + W[:, 0, :].astype(np.float64).sum(axis=0)
    ).astype(np.float32)
    bias2d = np.ascontiguousarray(bias_total.reshape(NU, 128).T)

    in_maps = []
    for i in range(NCORES):
        xt_i = np.ascontiguousarray(xT[:, i * BL : (i + 1) * BL])
        in_maps.append(
            {"xt": xt_i, "w12": w12b, "w3q": w3qb, "w4": w4b, "bias2d": bias2d}
        )

    if _NC_CACHE is None:
        _NC_CACHE = _build()
    nc = _NC_CACHE

    trace = bool(os.environ.get("KERNEL_TRACE"))
    res = run_bass_kernel_spmd(
        nc, in_maps, core_ids=list(range(NCORES)), trace=trace
    )
    LAST_EXEC_TIME_NS = res.exec_time_ns

    out = np.empty((B, U), dtype=np.float32)
    for i in range(NCORES):
        out[i * BL : (i + 1) * BL, :] = res.results[i]["out_t"].T
    return out


# revision 20
# speedup vs baseline: 3.5344x; 1.0102x over previous
"""KAN layer (polynomial basis) TRN2 kernel.

out = gelu(sum_{i,k} x[b,i]^k * W[i,k,j] + bias[j]),  exact gelu.
B=4096, D=1024, K=5, U=1024, fp32 I/O.

Strategy:
  - Data-parallel over batch: 8 cores x 512 rows each.
  - k=0 term (x^0=1) constant-folded on host into the bias:
    bias_total = bias + sum_i W[i,0,:].
  - x fed pre-transposed ([D, B_local]) so the contraction dim (D) lands
    on SBUF partitions; powers x^2..x^4 computed on-device.
  - Mixed-precision matmuls sized to the 2e-2 rel-err budget:
      k=1,2: both operands fp8 e4m3, fused into ONE DoubleRow matmul per
             (d,u) tile (2 fp8 weights/cell -> 0.5 cyc/row).
      k=3:   both operands fp8, with TWO ADJACENT d-chunks fused into one
             DoubleRow matmul (the pair rides the two DR slots), halving
             the k=3 cost again. x^3 is scaled by 1/2 so its e4m3 image
             stays under TRN's 240 cap (w3 scaled by 2S to compensate).
      k=4:   both operands fp16 (1 cyc/row; 11-bit mantissa keeps the
             x^4-term error negligible - it has the widest dynamic range).
    Weights pre-scaled by S=256 on host so fp8 W stays in e4m3 normal
    range; undone by the activation's scale=1/S.
    Measured rel err ~1.3e-2 vs the 2e-2 gate (dominated by the single-
    fp8 x^3 term; k=4 in fp8 as well would blow the budget).
  - Loop order d-outer/u-inner with all 8 PSUM banks as accumulators, so
    the PE consumes each x chunk right after its prep and never waits on
    a full prep pass.
  - Output computed transposed ([U, B_local]) so the per-unit bias is a
    per-partition scalar fused into the final Gelu; host transposes back.
"""

import os
import numpy as np
import ml_dtypes

from concourse import bacc
import concourse.mybir as mybir
import concourse.tile as tile
from concourse.bass_utils import run_bass_kernel_spmd

F32 = mybir.dt.float32
F16 = mybir.dt.float16
F8 = mybir.dt.float8e4
AF = mybir.ActivationFunctionType
DR = mybir.MatmulPerfMode.DoubleRow
MUL = mybir.AluOpType.mult

NCORES = 8
B, D, K, U = 4096, 1024, 5, 1024
BL = B // NCORES  # 512 batch rows per core
ND = D // 128  # 8 d chunks
NU = U // 128  # 8 u chunks
S = 256.0  # weight pre-scale (fp8 subnormal avoidance); undone in gelu

LAST_EXEC_TIME_NS = None


def _build():
    nc = bacc.Bacc("TRN2", target_bir_lowering=False, debug=False)
    xt = nc.dram_tensor("xt", [D, BL], F32, kind="ExternalInput").ap()
    w12 = nc.dram_tensor("w12", [ND, 128, 2, U], F8, kind="ExternalInput").ap()
    w3q = nc.dram_tensor("w3q", [ND // 2, 128, 2, U], F8, kind="ExternalInput").ap()
    w4 = nc.dram_tensor("w4", [ND, 128, U], F16, kind="ExternalInput").ap()
    bias2d = nc.dram_tensor("bias2d", [128, NU], F32, kind="ExternalInput").ap()
    out_t = nc.dram_tensor("out_t", [U, BL], F32, kind="ExternalOutput").ap()

    with tile.TileContext(nc) as tc:
        with (
            tc.tile_pool(name="xin", bufs=2) as xin,
            tc.tile_pool(name="wp", bufs=3) as wp,
            tc.tile_pool(name="xk", bufs=2) as xk,
            tc.tile_pool(name="tmp", bufs=2) as tmp,
            tc.tile_pool(name="op", bufs=4) as op,
            tc.tile_pool(name="res", bufs=1) as res,
            tc.tile_pool(name="ps", bufs=1, space="PSUM") as ps,
        ):
            pacc = [
                ps.tile([128, BL], F32, name=f"pacc{u}", tag=f"pacc{u}")
                for u in range(NU)
            ]

            bias_sb = res.tile([128, NU], F32, name="bias_sb")

            x4h_prev = None
            for d in range(ND):
                p, sl = d // 2, d % 2
                last = d == ND - 1
                # x on the scalar-engine HWDGE queue, weights on sync's —
                # parallel DMA issue (600ns per descriptor-gen each). The
                # very first x chunk is split across both queues to halve
                # its transfer latency (it gates the whole prolog).
                xf = xin.tile([128, BL], F32, name="xf", tag="xf")
                if d == 0:
                    nc.sync.dma_start(
                        xf[:, : BL // 2], xt[0:128, : BL // 2]
                    )
                    nc.scalar.dma_start(
                        xf[:, BL // 2 :], xt[0:128, BL // 2 :]
                    )
                else:
                    nc.scalar.dma_start(xf, xt[d * 128 : (d + 1) * 128, :])
                    if d == 1:
                        nc.scalar.dma_start(bias_sb, bias2d)
                w12t = wp.tile([128, 2, U], F8, name="w12t", tag="w12t")
                nc.sync.dma_start(w12t, w12[d])
                if sl == 0:
                    w4ta = wp.tile([128, U], F16, name="w4ta", tag="w4ta")
                    nc.sync.dma_start(w4ta, w4[d])
                    w3qt = wp.tile([128, 2, U], F8, name="w3qt", tag="w3qt")
                    nc.sync.dma_start(w3qt, w3q[p])
                else:
                    w4tb = wp.tile([128, U], F16, name="w4tb", tag="w4tb")
                    nc.sync.dma_start(w4tb, w4[d])

                # prep: DVE alone feeds the DR matmul (xq from xf only, no
                # cross-engine dep); ACT squares feed x^3 (fp8 pair slot)
                # and x^4 (fp16).
                xq = xk.tile([128, 2, BL], F8, name="xq", tag="xq")
                nc.vector.tensor_copy(xq[:, 0, :], xf)
                nc.vector.tensor_mul(out=xq[:, 1, :], in0=xf, in1=xf)
                x2f = tmp.tile([128, BL], F32, name="x2f", tag="x2f")
                nc.scalar.activation(x2f, xf, AF.Square)
                if sl == 0:
                    x3qp = xk.tile([128, 2, BL], F8, name="x3q", tag="x3q")
                nc.vector.scalar_tensor_tensor(
                    out=x3qp[:, sl, :], in0=x2f, scalar=0.5, in1=xf,
                    op0=MUL, op1=MUL,
                )
                x4h = xk.tile([128, BL], F16, name="x4h", tag="x4h")
                nc.scalar.activation(x4h, x2f, AF.Square)

                def mm_dr(u, wt, rhs, start=False, stop=False):
                    nc.tensor.matmul(
                        pacc[u], wt[:, :, u * 128 : (u + 1) * 128], rhs,
                        start=start, stop=stop, perf_mode=DR,
                    )

                def mm16(u, wt, rhs, stop=False):
                    nc.tensor.matmul(
                        pacc[u], wt[:, u * 128 : (u + 1) * 128], rhs,
                        start=False, stop=stop,
                    )

                if sl == 0:
                    # even chunk: just its k1/k2 DR matmuls; k3 waits for
                    # the pair, k4 waits a chunk so w4's DMA can land.
                    for u in range(NU):
                        mm_dr(u, w12t, xq, start=(d == 0))
                    x4h_prev, w4_prev = x4h, w4ta
                elif not last:
                    for u in range(NU):
                        mm16(u, w4_prev, x4h_prev)
                    for u in range(NU):
                        mm_dr(u, w12t, xq)
                    for u in range(NU):
                        mm_dr(u, w3qt, x3qp)
                    for u in range(NU):
                        mm16(u, w4tb, x4h)
                else:
                    # final chunk: interleave per-u (4 matmuls = 853ns per
                    # accumulator stop) so the 687ns gelus and their stores
                    # pipeline behind the mm stream instead of after it.
                    for u in range(NU):
                        mm16(u, w4_prev, x4h_prev)
                        mm_dr(u, w12t, xq)
                        mm_dr(u, w3qt, x3qp)
                        mm16(u, w4tb, x4h, stop=True)
                        osb = op.tile([128, BL], F32, name="osb", tag="osb")
                        nc.scalar.activation(
                            osb, pacc[u], AF.Gelu,
                            bias=bias_sb[:, u : u + 1], scale=1.0 / S,
                        )
                        nc.sync.dma_start(
                            out_t[u * 128 : (u + 1) * 128, :], osb
                        )

    nc.compile()
    return nc


_NC_CACHE = None


def kernel(x, basis_weights, bias):
    global _NC_CACHE, LAST_EXEC_TIME_NS
    x = np.asarray(x, dtype=np.float32)
    W = np.asarray(basis_weights, dtype=np.float32)
    bias = np.asarray(bias, dtype=np.float32)

    # ---- host prep: layout + dtype split + k=0 constant folding ----
    xT = np.ascontiguousarray(x.T)  # (D, B)
    Ws = W * np.float32(S)
    w12b = np.stack([Ws[:, 1, :], Ws[:, 2, :]], axis=1)  # (D, 2, U)
    w12b = np.ascontiguousarray(
        w12b.reshape(ND, 128, 2, U).astype(ml_dtypes.float8_e4m3)
    )
    w3qb = (2.0 * Ws[:, 3, :]).reshape(ND // 2, 2, 128, U).transpose(0, 2, 1, 3)
    w3qb = np.ascontiguousarray(w3qb.astype(ml_dtypes.float8_e4m3))
    w4b = np.ascontiguousarray(Ws[:, 4, :].reshape(ND, 128, U).astype(np.float16))
    bias_total = (
        bias.astype(np.float64) + W[:, 0, :].astype(np.float64).sum(axis=0)
    ).astype(np.float32)
    bias2d = np.ascontiguousarray(bias_total.reshape(NU, 128).T)

    in_maps = []
    for i in range(NCORES):
        xt_i = np.ascontiguousarray(xT[:, i * BL : (i + 1) * BL])
        in_maps.append(
            {"xt": xt_i, "w12": w12b, "w3q": w3qb, "w4": w4b, "bias2d": bias2d}
        )

    if _NC_CACHE is None:
        _NC_CACHE = _build()
    nc = _NC_CACHE

    trace = bool(os.environ.get("KERNEL_TRACE"))
    res = run_bass_kernel_spmd(
        nc, in_maps, core_ids=list(range(NCORES)), trace=trace
    )
    LAST_EXEC_TIME_NS = res.exec_time_ns

    out = np.empty((B, U), dtype=np.float32)
    for i in range(NCORES):
        out[i * BL : (i + 1) * BL, :] = res.results[i]["out_t"].T
    return out


# revision 21
# speedup vs baseline: 3.7042x; 1.0480x over previous
"""KAN layer (polynomial basis) TRN2 kernel.

out = gelu(sum_{i,k} x[b,i]^k * W[i,k,j] + bias[j]),  exact gelu.
B=4096, D=1024, K=5, U=1024, fp32 I/O.

Strategy:
  - Data-parallel over batch: 8 cores x 512 rows each.
  - k=0 term (x^0=1) constant-folded on host into the bias:
    bias_total = bias + sum_i W[i,0,:].
  - x fed pre-transposed ([D, B_local]) so the contraction dim (D) lands
    on SBUF partitions; powers x^2..x^4 computed on-device.
  - Mixed-precision matmuls sized to the 2e-2 rel-err budget:
      k=1,2: both operands fp8 e4m3, fused into ONE DoubleRow matmul per
             (d,u) tile (2 fp8 weights/cell -> 0.5 cyc/row).
      k=3:   both operands fp8, with TWO ADJACENT d-chunks fused into one
             DoubleRow matmul (the pair rides the two DR slots), halving
             the k=3 cost again. x^3 is scaled by 1/2 so its e4m3 image
             stays under TRN's 240 cap (w3 scaled by 2S to compensate).
      k=4:   both operands fp16 (1 cyc/row; 11-bit mantissa keeps the
             x^4-term error negligible - it has the widest dynamic range).
    Weights pre-scaled by S=256 on host so fp8 W stays in e4m3 normal
    range; undone by the activation's scale=1/S.
    Measured rel err ~1.3e-2 vs the 2e-2 gate (dominated by the single-
    fp8 x^3 term; k=4 in fp8 as well would blow the budget).
  - Loop order d-outer/u-inner with all 8 PSUM banks as accumulators, so
    the PE consumes each x chunk right after its prep and never waits on
    a full prep pass.
  - Output computed transposed ([U, B_local]) so the per-unit bias is a
    per-partition scalar fused into the final Gelu; host transposes back.
"""

import os
import numpy as np
import ml_dtypes

from concourse import bacc
import concourse.mybir as mybir
import concourse.tile as tile
from concourse.bass_utils import run_bass_kernel_spmd

F32 = mybir.dt.float32
F16 = mybir.dt.float16
F8 = mybir.dt.float8e4
AF = mybir.ActivationFunctionType
DR = mybir.MatmulPerfMode.DoubleRow
MUL = mybir.AluOpType.mult

NCORES = 8
B, D, K, U = 4096, 1024, 5, 1024
BL = B // NCORES  # 512 batch rows per core
ND = D // 128  # 8 d chunks
NU = U // 128  # 8 u chunks
S = 256.0  # weight pre-scale (fp8 subnormal avoidance); undone in gelu

LAST_EXEC_TIME_NS = None


def _build():
    nc = bacc.Bacc("TRN2", target_bir_lowering=False, debug=False)
    xt = nc.dram_tensor("xt", [D, BL], F16, kind="ExternalInput").ap()
    w12 = nc.dram_tensor("w12", [ND, 128, 2, U], F8, kind="ExternalInput").ap()
    w3q = nc.dram_tensor("w3q", [ND // 2, 128, 2, U], F8, kind="ExternalInput").ap()
    w4 = nc.dram_tensor("w4", [ND, 128, U], F16, kind="ExternalInput").ap()
    bias2d = nc.dram_tensor("bias2d", [128, NU], F32, kind="ExternalInput").ap()
    out_t = nc.dram_tensor("out_t", [U, BL], F32, kind="ExternalOutput").ap()

    with tile.TileContext(nc) as tc:
        with (
            tc.tile_pool(name="xin", bufs=3) as xin,
            tc.tile_pool(name="wp", bufs=4) as wp,
            tc.tile_pool(name="xk", bufs=3) as xk,
            tc.tile_pool(name="tmp", bufs=3) as tmp,
            tc.tile_pool(name="op", bufs=4) as op,
            tc.tile_pool(name="res", bufs=1) as res,
            tc.tile_pool(name="ps", bufs=1, space="PSUM") as ps,
        ):
            pacc = [
                ps.tile([128, BL], F32, name=f"pacc{u}", tag=f"pacc{u}")
                for u in range(NU)
            ]

            bias_sb = res.tile([128, NU], F32, name="bias_sb")

            x4h_prev = None
            for d in range(ND):
                p, sl = d // 2, d % 2
                last = d == ND - 1
                # x on the scalar-engine HWDGE queue, weights on sync's —
                # parallel DMA issue (600ns per descriptor-gen each). The
                # very first x chunk is split across both queues to halve
                # its transfer latency (it gates the whole prolog).
                xf = xin.tile([128, BL], F16, name="xf", tag="xf")
                nc.scalar.dma_start(xf, xt[d * 128 : (d + 1) * 128, :])
                if d == 1:
                    nc.scalar.dma_start(bias_sb, bias2d)
                w12t = wp.tile([128, 2, U], F8, name="w12t", tag="w12t")
                nc.sync.dma_start(w12t, w12[d])
                if sl == 0:
                    w4ta = wp.tile([128, U], F16, name="w4ta", tag="w4ta")
                    nc.sync.dma_start(w4ta, w4[d])
                    w3qt = wp.tile([128, 2, U], F8, name="w3qt", tag="w3qt")
                    nc.sync.dma_start(w3qt, w3q[p])
                else:
                    w4tb = wp.tile([128, U], F16, name="w4tb", tag="w4tb")
                    nc.sync.dma_start(w4tb, w4[d])

                # prep: DVE alone feeds the DR matmul (xq from xf only, no
                # cross-engine dep); ACT squares feed x^3 (fp8 pair slot)
                # and x^4 (fp16).
                xq = xk.tile([128, 2, BL], F8, name="xq", tag="xq")
                nc.vector.tensor_copy(xq[:, 0, :], xf)
                nc.vector.tensor_mul(out=xq[:, 1, :], in0=xf, in1=xf)
                x2f = tmp.tile([128, BL], F32, name="x2f", tag="x2f")
                nc.scalar.activation(x2f, xf, AF.Square)
                if sl == 0:
                    x3qp = xk.tile([128, 2, BL], F8, name="x3q", tag="x3q")
                nc.vector.scalar_tensor_tensor(
                    out=x3qp[:, sl, :], in0=x2f, scalar=0.5, in1=xf,
                    op0=MUL, op1=MUL,
                )
                x4h = xk.tile([128, BL], F16, name="x4h", tag="x4h")
                nc.scalar.activation(x4h, x2f, AF.Square)

                def mm_dr(u, wt, rhs, start=False, stop=False):
                    nc.tensor.matmul(
                        pacc[u], wt[:, :, u * 128 : (u + 1) * 128], rhs,
                        start=start, stop=stop, perf_mode=DR,
                    )

                def mm16(u, wt, rhs, stop=False):
                    nc.tensor.matmul(
                        pacc[u], wt[:, u * 128 : (u + 1) * 128], rhs,
                        start=False, stop=stop,
                    )

                if sl == 0:
                    # even chunk: just its k1/k2 DR matmuls; k3 waits for
                    # the pair, k4 waits a chunk so w4's DMA can land.
                    for u in range(NU):
                        mm_dr(u, w12t, xq, start=(d == 0))
                    x4h_prev, w4_prev = x4h, w4ta
                elif not last:
                    for u in range(NU):
                        mm16(u, w4_prev, x4h_prev)
                    for u in range(NU):
                        mm_dr(u, w12t, xq)
                    for u in range(NU):
                        mm_dr(u, w3qt, x3qp)
                    for u in range(NU):
                        mm16(u, w4tb, x4h)
                else:
                    # final chunk: interleave per-u (4 matmuls = 853ns per
                    # accumulator stop) so the 687ns gelus and their stores
                    # pipeline behind the mm stream instead of after it.
                    for u in range(NU):
                        mm16(u, w4_prev, x4h_prev)
                        mm_dr(u, w12t, xq)
                        mm_dr(u, w3qt, x3qp)
                        mm16(u, w4tb, x4h, stop=True)
                        osb = op.tile([128, BL], F32, name="osb", tag="osb")
                        nc.scalar.activation(
                            osb, pacc[u], AF.Gelu,
                            bias=bias_sb[:, u : u + 1], scale=1.0 / S,
                        )
                        nc.sync.dma_start(
                            out_t[u * 128 : (u + 1) * 128, :], osb
                        )

    nc.compile()
    return nc


_NC_CACHE = None


def kernel(x, basis_weights, bias):
    global _NC_CACHE, LAST_EXEC_TIME_NS
    x = np.asarray(x, dtype=np.float32)
    W = np.asarray(basis_weights, dtype=np.float32)
    bias = np.asarray(bias, dtype=np.float32)

    # ---- host prep: layout + dtype split + k=0 constant folding ----
    xT = np.ascontiguousarray(x.T.astype(np.float16))  # (D, B)
    Ws = W * np.float32(S)
    w12b = np.stack([Ws[:, 1, :], Ws[:, 2, :]], axis=1)  # (D, 2, U)
    w12b = np.ascontiguousarray(
        w12b.reshape(ND, 128, 2, U).astype(ml_dtypes.float8_e4m3)
    )
    w3qb = (2.0 * Ws[:, 3, :]).reshape(ND // 2, 2, 128, U).transpose(0, 2, 1, 3)
    w3qb = np.ascontiguousarray(w3qb.astype(ml_dtypes.float8_e4m3))
    w4b = np.ascontiguousarray(Ws[:, 4, :].reshape(ND, 128, U).astype(np.float16))
    bias_total = (
        bias.astype(np.float64) + W[:, 0, :].astype(np.float64).sum(axis=0)
    ).astype(np.float32)
    bias2d = np.ascontiguousarray(bias_total.reshape(NU, 128).T)

    in_maps = []
    for i in range(NCORES):
        xt_i = np.ascontiguousarray(xT[:, i * BL : (i + 1) * BL])
        in_maps.append(
            {"xt": xt_i, "w12": w12b, "w3q": w3qb, "w4": w4b, "bias2d": bias2d}
        )

    if _NC_CACHE is None:
        _NC_CACHE = _build()
    nc = _NC_CACHE

    trace = bool(os.environ.get("KERNEL_TRACE"))
    res = run_bass_kernel_spmd(
        nc, in_maps, core_ids=list(range(NCORES)), trace=trace
    )
    LAST_EXEC_TIME_NS = res.exec_time_ns

    out = np.empty((B, U), dtype=np.float32)
    for i in range(NCORES):
        out[i * BL : (i + 1) * BL, :] = res.results[i]["out_t"].T
    return out
